# revision 64
# baseline (speedup 1.0000x reference)
"""Trainium2 Bass kernel for NonLinearSelfAttention.

Computes, per batch b:
    S    = x_b @ x_b.T * C**-0.5          [N, N]
    P    = softmax(S, axis=-1)
    out  = (P @ x_b) @ W.T + bias         [N, OUT]

Sharding: batch-data-parallel, one batch per NeuronCore (8 cores).

Per-core algorithm (N=4096, C=128):
  - E = exp(scale*S) is symmetric, so the tile E[J-block, A-block] computed in
    [j, i] layout is directly the lhsT needed by the P@V matmul for output
    block A — no transposes in the main loop.
  - The Linear folds through the attention: y = (E @ [z | 1]) / r + bias with
    z = x @ W.T, because (P x) W.T = P (x W.T).  The appended ones column
    produces the softmax row-sums r in per-partition layout for free
    (r_i = sum_j E[j, i] = sum_j E[i, j] by symmetry).
  - No max-subtraction needed: logits are ~N(0,1) with diagonal ~ sqrt(C)+,
    max ~ 20, exp(20) is well inside fp32 range.
  - exp is split between ScalarE (native, exact — always covers the diagonal
    128-blocks, which carry ~92% of the softmax mass) and VectorE
    (Schraudolph bit-trick: bf16 bits of e^u ~= round(u*K1 + K2), computed
    as one tensor_scalar with int16 output and bitcast to bf16; ~4% worst
    pointwise on the off-diagonal mass, ~3.5e-3 end-to-end).  A running
    busy-ns estimate splits each slab's free range between the engines.
  - Sequence positions are permuted (tile T, partition p holds row p*NT+T)
    so the x load is contiguous per partition; attention is permutation-
    equivariant, and only the output DMA un-permutes.
  - x DMA chunks are striped across both HWDGE rings (sync + scalar).
"""
import numpy as np

import concourse.bass as bass
import concourse.tile as tile
from concourse.masks import make_identity
from concourse import bacc, mybir
from concourse import bass_utils

B = 8          # batches = cores
N = 4096       # sequence length
C = 128        # feature dim
OUT = 128      # linear out dim
NT = N // 128  # 32 j-tiles
QW = 512       # i-columns processed per quad-block
NQ = N // QW   # 8 quad blocks
SCALE = float(C) ** -0.5

SCALE_ADJ = SCALE
LOG2E = 1.4426950408889634
EXP_K1 = SCALE_ADJ * LOG2E * 128.0
EXP_K2 = 16256.0 - 7.2192745       # 127<<7 + c (c: zero-mean rel err)

F32 = mybir.dt.float32
BF16 = mybir.dt.bfloat16
I16 = mybir.dt.int16
U16 = mybir.dt.uint16


def _build(ctx_dtype=BF16):
    nc = bacc.Bacc("TRN2", target_bir_lowering=False, debug=False, num_devices=B)
    x_d = nc.dram_tensor("x", [N, C], F32, kind="ExternalInput").ap()
    w_d = nc.dram_tensor("W", [OUT, C], F32, kind="ExternalInput").ap()
    b_d = nc.dram_tensor("b", [OUT], F32, kind="ExternalInput").ap()
    o_d = nc.dram_tensor("out", [N, OUT], F32, kind="ExternalOutput").ap()

    with tile.TileContext(nc) as tc:
        with tc.tile_pool(name="const", bufs=1) as const, \
             tc.tile_pool(name="bwork", bufs=7) as bwork, \
             tc.tile_pool(name="ywork", bufs=2) as ywork, \
             tc.tile_pool(name="ps_work", bufs=3, space="PSUM") as ps_work, \
             tc.tile_pool(name="ps_acc", bufs=2, space="PSUM") as ps_acc:

            # running estimate of each exp engine's busy-ns, used to split
            # slabs; DVE also carries z-copies and the per-quad epilogue
            bal = {"s": 0.0, "d": 0.0}

            # ---- setup ----
            # x loads: few big DMAs (each InstDMACopy splits across all 16
            # SDMA slots; many small DMAs pay ~600ns serial issue each)
            # Sequence positions are PERMUTED: tile T, partition p holds row
            # p*NT + T.  Attention is permutation-equivariant (row softmax +
            # both einsum sides carry the same permutation), so the math is
            # unchanged; only the output DMA un-permutes.  This makes the x
            # load contiguous per partition (16KB runs -> full HBM bandwidth)
            # instead of 512B granules (~2x faster load).
            x_nat = const.tile([128, NT, 128], F32)       # x tiles [j within tile, c]
            x_view = x_d.rearrange("(p t) c -> p t c", t=NT)
            # few DMA chunks (each dma_start pays ~600ns serial issue), small
            # leading ones so the cast/transpose pipeline starts early; the
            # cast/transpose/copy pipeline below is demand-driven per 4-tile
            # group, so the DVE never head-of-line-blocks an early xT copy
            # behind a late cast
            # stripe chunks across BOTH physical HWDGE rings (sync=qSPDynamicHW,
            # scalar=qActDynamicHW) — a single ring tops out ~165GB/s at these
            # chunk sizes; two rings in flight roughly double the load rate
            # small leading chunks striped across the rings for an early
            # pipeline start, then one big chunk for HBM efficiency
            bounds = [0, 4, 10, 16, NT]
            for ci, (lo, hi) in enumerate(zip(bounds, bounds[1:])):
                eng = nc.sync if ci % 2 == 0 else nc.scalar
                eng.dma_start(x_nat[:, lo:hi, :], x_view[:, lo:hi, :])

            w_sb = const.tile([128, 128], F32)            # W [o, c]
            nc.scalar.dma_start(w_sb, w_d)
            bias_bc = const.tile([128, 128], F32)         # bias broadcast to all partitions
            nc.scalar.dma_start(bias_bc, bass.AP(tensor=b_d.tensor, offset=b_d.offset,
                                                 ap=[[0, 128]] + b_d.ap))

            x_bf = const.tile([128, NT, 128], BF16)
            # all casts upfront: the scheduler runs each as soon as its DMA
            # chunk lands (ready-order beats priority), so they never queue
            # behind main-loop exp work on the DVE
            for g in range(NT // 4):
                nc.vector.tensor_copy(x_bf[:, g * 4:(g + 1) * 4, :],
                                      x_nat[:, g * 4:(g + 1) * 4, :])
                bal["d"] += (58 + 256) / 0.96

            ident = const.tile([128, 128], BF16)
            make_identity(nc, ident)
            xT = const.tile([128, N], BF16)               # [c, n]

            # small first transpose groups so quad 0 starts early; big late
            # groups so fewer PSUM pool insertions pace the late xT copies
            XT_STARTS = [0, 4, 8, 16, 24, NT]

            def emit_xT_group(gi):
                t0, t1 = XT_STARTS[gi], XT_STARTS[gi + 1]
                t_ps = ps_work.tile([128, (t1 - t0) * 128], BF16,
                                    name="t_ps", tag="pswork")
                for u in range(t1 - t0):
                    nc.tensor.transpose(t_ps[:, u * 128:(u + 1) * 128],
                                        x_bf[:, t0 + u, :], ident)
                # high priority: this copy frees a ps_work generation the
                # slab stream is waiting on — don't let it queue behind exp
                with tc.high_priority():
                    nc.vector.tensor_copy(xT[:, t0 * 128:t1 * 128], t_ps)
                bal["d"] += (120 + (t1 - t0) * 64) / 0.96

            xT_state = {"emitted": 0}

            def ensure_xT(j_hi):
                while xT_state["emitted"] < len(XT_STARTS) - 1 and \
                        XT_STARTS[xT_state["emitted"]] < j_hi:
                    emit_xT_group(xT_state["emitted"])
                    xT_state["emitted"] += 1

            w_bf = const.tile([128, 128], BF16)
            nc.vector.tensor_copy(w_bf, w_sb)
            ensure_xT(4)  # group 0: quad 0's rhs columns
            wt_ps = ps_work.tile([128, 512], BF16, name="t_ps", tag="pswork")
            nc.tensor.transpose(wt_ps[:, 0:128], w_bf, ident)
            wT = const.tile([128, 128], BF16)             # wT[c, o] = W[o, c]
            nc.vector.tensor_copy(wT, wt_ps[:, 0:128])

            # z~ = [x @ W.T | 1]  (bf16), tiled [j within tile, 129]
            zt = const.tile([128, NT, 129], ctx_dtype)
            nc.vector.memset(zt[:, :, 128], 1.0)

            def emit_z_group(g):
                # 8 j-tiles per group: halves the PSUM pool insertions and
                # the copy count (pool rotation during quad 0 also paces the
                # xT transposes, so fewer insertions help both)
                z_ps = ps_work.tile([128, 1024], F32, name="z_ps", tag="pswork")
                for u in range(8):
                    j = g * 8 + u
                    nc.tensor.matmul(z_ps[:, u * 128:(u + 1) * 128],
                                     xT[:, j * 128:(j + 1) * 128], wT,
                                     start=True, stop=True)
                # fold the bias into z:  sum_j E_ij (z_jo + b_o) =
                # num_io + r_i*b_o, so y = num'/r needs no bias add later
                bias_row = bass.AP(tensor=bias_bc.tensor, offset=bias_bc.offset,
                                   ap=[bias_bc.ap[0], [0, 8], bias_bc.ap[-1]])
                nc.vector.tensor_tensor(
                    zt[:, g * 8:(g + 1) * 8, 0:128],
                    z_ps.rearrange("p (j c) -> p j c", c=128),
                    bias_row, op=mybir.AluOpType.add)
                bal["d"] += (120 + 1024) / 0.96

            z_state = {"emitted": 0}

            def ensure_z(j_hi):
                need = min(NT // 8, (j_hi + 7) // 8)
                while z_state["emitted"] < need:
                    emit_z_group(z_state["emitted"])
                    z_state["emitted"] += 1

            zeros128 = const.tile([128, 128], ctx_dtype)
            nc.vector.memset(zeros128, 0.0)
            dummy258 = const.tile([128, 258], ctx_dtype)
            nc.vector.memset(dummy258, 0.0)


            # prefetch a couple of xT/z groups so quad 0's pipeline starts deep
            ensure_xT(8)
            ensure_z(4)

            # ---- main loop ----
            # Non-diagonal S slabs are written by the PE directly as BF16
            # into single-bank PSUM tiles: halves PSUM footprint (4 slabs in
            # flight) and doubles the DVE bit-trick rate (2x_1P on 16-bit
            # PSUM reads).  Diagonal j-tiles keep fp32 slabs and exact
            # ScalarE exp (~92% of softmax mass is on the diagonal).
            # The four acc accumulators pack two-per-bank: a zero matmul
            # opens the bank's accumulation group (start=True clears
            # has_written bank-wide), then every AV matmul accumulates with
            # start=False.  S-matmuls are emitted three slabs AHEAD so they
            # sit in front of in-flight AVs in the PE FIFO.
            JG = [2] * 16             # j-block group sizes per quad (sum=32);
                                      # 2-bank slabs, 3 in flight (bufs=3),
                                      # so exp latency jitter never stalls PE
            NB = QW // 128            # i-blocks per quad (4)
            groups = []
            for q in range(NQ):
                jb = 0
                for hi, gsz in enumerate(JG):
                    groups.append((q, jb, gsz, hi))
                    jb += gsz

            s_tiles = {}

            def emit_S(idx):
                q, jb, gsz, hi = groups[idx]
                ensure_xT(jb + gsz)
                s_ps = ps_work.tile([128, QW * gsz], F32, name="s_ps",
                                    tag="pswork")
                for u in range(gsz):
                    j = jb + u
                    nc.tensor.matmul(s_ps[:, u * QW:(u + 1) * QW],
                                     xT[:, j * 128:(j + 1) * 128],
                                     xT[:, q * QW:(q + 1) * QW],
                                     start=True, stop=True)
                s_tiles[idx] = s_ps

            def diag_span(q, jb, gsz):
                lo = hi = None
                for u in range(gsz):
                    j = jb + u
                    if 4 * q <= j < 4 * q + 4:
                        off = u * QW + (j - 4 * q) * 128
                        lo = off if lo is None else lo
                        hi = off + 128
                return lo, hi

            def emit_exp(s_ps, b_sb, q, jb, gsz):
                """Assign each slab's exp wholly to ScalarE (native exp) or
                wholly to the VectorE bit-trick, alternating via the running
                busy estimate (whole slabs halve per-instruction overhead vs
                always splitting); slabs containing diagonal blocks keep the
                in-slab split with ScalarE forced over the diagonal span."""
                FD = gsz * QW
                dlo, dhi = diag_span(q, jb, gsz)
                b_i16 = b_sb.bitcast(I16)
                # cap the scalar segment so its latency (~(350+FD)/1.2 ns)
                # stays inside the 882ns/slab PE period; diagonal spans may
                # exceed the cap when forced
                cap = 640 if dlo is None else max(640, dhi - dlo)
                cands = [(a, b2) for a in range(0, FD + 128, 128)
                         for b2 in range(a, FD + 128, 128)
                         if (b2 - a) <= cap
                         and (dlo is None or (a <= dlo and b2 >= dhi))]
                best = None
                for a, b2 in cands:
                    ts = bal["s"] + ((350 + (b2 - a)) / 1.2 if b2 > a else 0.0)
                    td = bal["d"] + sum((120 + (h - l)) / 0.96
                                        for l, h in ((0, a), (b2, FD)) if h > l)
                    m = max(ts, td)
                    if best is None or m < best[0]:
                        best = (m, a, b2)
                _, a, b2 = best
                if b2 > a:
                    nc.scalar.activation(b_sb[:, a:b2], s_ps[:, a:b2],
                                         mybir.ActivationFunctionType.Exp,
                                         scale=SCALE_ADJ)
                    bal["s"] += (350 + (b2 - a)) / 1.2
                for l, h in ((0, a), (b2, FD)):
                    if h > l:
                        nc.vector.tensor_scalar(b_i16[:, l:h], s_ps[:, l:h],
                                                EXP_K1, EXP_K2,
                                                op0=mybir.AluOpType.mult,
                                                op1=mybir.AluOpType.add)
                        bal["d"] += (120 + (h - l)) / 0.96

            emit_S(0)
            emit_S(1)
            acc = None
            acc_slice = None
            for idx, (q, jb, gsz, hi) in enumerate(groups):
                if hi == 0:
                    acc = [ps_acc.tile([128, 258], F32, name=f"acc{p}",
                                       tag="acc")
                           for p in range(NB // 2)]

                    def acc_slice(k, w=129, _acc=acc):
                        return _acc[k // 2][:, (k % 2) * 129:(k % 2) * 129 + w]

                if idx + 2 < len(groups):
                    emit_S(idx + 2)
                s_ps = s_tiles.pop(idx)
                b_sb = bwork.tile([128, QW * gsz], ctx_dtype, name="b_sb",
                                  tag="b_sb")
                emit_exp(s_ps, b_sb, q, jb, gsz)
                ensure_z(jb + gsz)
                for u in range(gsz):
                    j = jb + u
                    for k in range(NB):
                        # first MM into each acc bank per quad opens the
                        # accumulation group: start=True clears has_written
                        # bank-wide, the paired k (odd) then overwrites its
                        # untouched columns, and all later j accumulate.
                        # MMs complete in FIFO order, so this is safe.
                        nc.tensor.matmul(
                            acc_slice(k),
                            b_sb[:, u * QW + k * 128:u * QW + (k + 1) * 128],
                            zt[:, j, :],
                            start=(hi == 0 and u == 0 and k % 2 == 0),
                            stop=(j == NT - 1),
                            skip_group_check=True)
                if hi != len(JG) - 1:
                    continue
                # epilogue: y = acc[:, :128] / acc[:, 128]  (bias rides in z).
                # The two accs packed per bank sit at stride 129, so one
                # strided reciprocal + one tensor_tensor multiply handles a
                # pair; on the final quad half the scales run on ScalarE so
                # the drain tail isn't serialized through the DVE queue.
                y4 = ywork.tile([128, NB, 128], F32, name="y4", tag="y4")
                for p in range(NB // 2):
                    base = acc[p]
                    rinv2 = ywork.tile([128, 2], F32, name="rinv", tag="rinv")
                    r_pair = bass.AP(tensor=base.tensor, offset=base.offset + 128,
                                     ap=[base.ap[0], [129, 2]])
                    nc.vector.reciprocal(rinv2, r_pair)
                    if q == NQ - 1 and p == 1:
                        for kk in range(2):
                            nc.scalar.activation(
                                y4[:, 2 * p + kk, :], acc_slice(2 * p + kk, 128),
                                mybir.ActivationFunctionType.Copy,
                                scale=rinv2[:, kk:kk + 1])
                    else:
                        num_pair = bass.AP(tensor=base.tensor, offset=base.offset,
                                           ap=[base.ap[0], [129, 2], [1, 128]])
                        rinv_bc = bass.AP(tensor=rinv2.tensor, offset=rinv2.offset,
                                          ap=[rinv2.ap[0], [1, 2], [0, 128]])
                        nc.vector.tensor_tensor(y4[:, 2 * p:2 * p + 2, :],
                                                num_pair, rinv_bc,
                                                op=mybir.AluOpType.mult)
                bal["d"] += 1100.0
                o_view = o_d.rearrange("(p m) c -> p m c", m=NT)
                if q == NQ - 1:
                    # final quad is on the drain critical path: one DMA per
                    # i-block, striped over both HWDGE rings, so the first
                    # writes start while later epilogue ops still run
                    for k in range(NB):
                        eng = nc.sync if k % 2 == 0 else nc.scalar
                        eng.dma_start(o_view[:, q * NB + k, :], y4[:, k, :])
                else:
                    nc.sync.dma_start(o_view[:, q * NB:(q + 1) * NB, :], y4)

    nc.compile()
    return nc


_NC_CACHE = {}


def _get_nc():
    if "nc" not in _NC_CACHE:
        _NC_CACHE["nc"] = _build()
    return _NC_CACHE["nc"]


def kernel(x, W, b, _trace=False):
    """x: [8, 4096, 128] f32, W: [128, 128] f32, b: [128] f32 -> [8, 4096, 128] f32."""
    nc = _get_nc()
    x = np.ascontiguousarray(np.asarray(x, dtype=np.float32))
    W = np.ascontiguousarray(np.asarray(W, dtype=np.float32))
    b = np.ascontiguousarray(np.asarray(b, dtype=np.float32))
    in_maps = [{"x": x[i], "W": W, "b": b} for i in range(B)]
    res = bass_utils.run_bass_kernel_spmd(nc, in_maps, core_ids=list(range(B)),
                                          trace=_trace)
    out = np.stack([r["out"] for r in res.results]).astype(np.float32)
    if _trace:
        return out, res
    return out


# revision 65
# speedup vs baseline: 1.0158x; 1.0158x over previous
"""Trainium2 Bass kernel for NonLinearSelfAttention.

Computes, per batch b:
    S    = x_b @ x_b.T * C**-0.5          [N, N]
    P    = softmax(S, axis=-1)
    out  = (P @ x_b) @ W.T + bias         [N, OUT]

Sharding: batch-data-parallel, one batch per NeuronCore (8 cores).

Per-core algorithm (N=4096, C=128):
  - E = exp(scale*S) is symmetric, so the tile E[J-block, A-block] computed in
    [j, i] layout is directly the lhsT needed by the P@V matmul for output
    block A — no transposes in the main loop.
  - The Linear folds through the attention: y = (E @ [z | 1]) / r + bias with
    z = x @ W.T, because (P x) W.T = P (x W.T).  The appended ones column
    produces the softmax row-sums r in per-partition layout for free
    (r_i = sum_j E[j, i] = sum_j E[i, j] by symmetry).
  - No max-subtraction needed: logits are ~N(0,1) with diagonal ~ sqrt(C)+,
    max ~ 20, exp(20) is well inside fp32 range.
  - exp is split between ScalarE (native, exact — always covers the diagonal
    128-blocks, which carry ~92% of the softmax mass) and VectorE
    (Schraudolph bit-trick: bf16 bits of e^u ~= round(u*K1 + K2), computed
    as one tensor_scalar with int16 output and bitcast to bf16; ~4% worst
    pointwise on the off-diagonal mass, ~3.5e-3 end-to-end).  A running
    busy-ns estimate splits each slab's free range between the engines.
  - Sequence positions are permuted (tile T, partition p holds row p*NT+T)
    so the x load is contiguous per partition; attention is permutation-
    equivariant, and only the output DMA un-permutes.
  - x DMA chunks are striped across both HWDGE rings (sync + scalar).
"""
import numpy as np

import concourse.bass as bass
import concourse.tile as tile
from concourse.masks import make_identity
from concourse import bacc, mybir
from concourse import bass_utils

B = 8          # batches = cores
N = 4096       # sequence length
C = 128        # feature dim
OUT = 128      # linear out dim
NT = N // 128  # 32 j-tiles
QW = 512       # i-columns processed per quad-block
NQ = N // QW   # 8 quad blocks
SCALE = float(C) ** -0.5

SCALE_ADJ = SCALE
LOG2E = 1.4426950408889634
EXP_K1 = SCALE_ADJ * LOG2E * 128.0
EXP_K2 = 16256.0 - 7.2192745       # 127<<7 + c (c: zero-mean rel err)

F32 = mybir.dt.float32
BF16 = mybir.dt.bfloat16
I16 = mybir.dt.int16
U16 = mybir.dt.uint16


def _build(ctx_dtype=BF16):
    nc = bacc.Bacc("TRN2", target_bir_lowering=False, debug=False, num_devices=B)
    x_d = nc.dram_tensor("x", [N, C], F32, kind="ExternalInput").ap()
    w_d = nc.dram_tensor("W", [OUT, C], F32, kind="ExternalInput").ap()
    b_d = nc.dram_tensor("b", [OUT], F32, kind="ExternalInput").ap()
    o_d = nc.dram_tensor("out", [N, OUT], F32, kind="ExternalOutput").ap()

    with tile.TileContext(nc) as tc:
        with tc.tile_pool(name="const", bufs=1) as const, \
             tc.tile_pool(name="bwork", bufs=7) as bwork, \
             tc.tile_pool(name="ywork", bufs=2) as ywork, \
             tc.tile_pool(name="ps_work", bufs=3, space="PSUM") as ps_work, \
             tc.tile_pool(name="ps_acc", bufs=2, space="PSUM") as ps_acc:

            # running estimate of each exp engine's busy-ns, used to split
            # slabs; DVE also carries z-copies and the per-quad epilogue
            bal = {"s": 0.0, "d": 0.0}

            # ---- setup ----
            # x loads: few big DMAs (each InstDMACopy splits across all 16
            # SDMA slots; many small DMAs pay ~600ns serial issue each)
            # Sequence positions are PERMUTED: tile T, partition p holds row
            # p*NT + T.  Attention is permutation-equivariant (row softmax +
            # both einsum sides carry the same permutation), so the math is
            # unchanged; only the output DMA un-permutes.  This makes the x
            # load contiguous per partition (16KB runs -> full HBM bandwidth)
            # instead of 512B granules (~2x faster load).
            x_nat = const.tile([128, NT, 128], F32)       # x tiles [j within tile, c]
            x_view = x_d.rearrange("(p t) c -> p t c", t=NT)
            # few DMA chunks (each dma_start pays ~600ns serial issue), small
            # leading ones so the cast/transpose pipeline starts early; the
            # cast/transpose/copy pipeline below is demand-driven per 4-tile
            # group, so the DVE never head-of-line-blocks an early xT copy
            # behind a late cast
            # stripe chunks across BOTH physical HWDGE rings (sync=qSPDynamicHW,
            # scalar=qActDynamicHW) — a single ring tops out ~165GB/s at these
            # chunk sizes; two rings in flight roughly double the load rate
            # small leading chunks striped across the rings for an early
            # pipeline start, then one big chunk for HBM efficiency
            # W/bias FIRST on the scalar ring: they are tiny but gate wT ->
            # z-group 0 -> the first AV matmuls; queued after the big x
            # chunk they would not land until ~17us
            w_sb = const.tile([128, 128], F32)            # W [o, c]
            nc.scalar.dma_start(w_sb, w_d)
            bias_bc = const.tile([128, 128], F32)         # bias broadcast to all partitions
            nc.scalar.dma_start(bias_bc, bass.AP(tensor=b_d.tensor, offset=b_d.offset,
                                                 ap=[[0, 128]] + b_d.ap))

            bounds = [0, 4, 10, 16, NT]
            for ci, (lo, hi) in enumerate(zip(bounds, bounds[1:])):
                eng = nc.sync if ci % 2 == 0 else nc.scalar
                eng.dma_start(x_nat[:, lo:hi, :], x_view[:, lo:hi, :])

            x_bf = const.tile([128, NT, 128], BF16)
            # all casts upfront: the scheduler runs each as soon as its DMA
            # chunk lands (ready-order beats priority), so they never queue
            # behind main-loop exp work on the DVE
            for g in range(NT // 4):
                nc.vector.tensor_copy(x_bf[:, g * 4:(g + 1) * 4, :],
                                      x_nat[:, g * 4:(g + 1) * 4, :])
                bal["d"] += (58 + 256) / 0.96

            ident = const.tile([128, 128], BF16)
            make_identity(nc, ident)
            xT = const.tile([128, N], BF16)               # [c, n]

            # small first transpose groups so quad 0 starts early; big late
            # groups so fewer PSUM pool insertions pace the late xT copies
            XT_STARTS = [0, 4, 8, 16, 24, NT]

            def emit_xT_group(gi):
                t0, t1 = XT_STARTS[gi], XT_STARTS[gi + 1]
                t_ps = ps_work.tile([128, (t1 - t0) * 128], BF16,
                                    name="t_ps", tag="pswork")
                for u in range(t1 - t0):
                    nc.tensor.transpose(t_ps[:, u * 128:(u + 1) * 128],
                                        x_bf[:, t0 + u, :], ident)
                # high priority: this copy frees a ps_work generation the
                # slab stream is waiting on — don't let it queue behind exp
                with tc.high_priority():
                    nc.vector.tensor_copy(xT[:, t0 * 128:t1 * 128], t_ps)
                bal["d"] += (120 + (t1 - t0) * 64) / 0.96

            xT_state = {"emitted": 0}

            def ensure_xT(j_hi):
                while xT_state["emitted"] < len(XT_STARTS) - 1 and \
                        XT_STARTS[xT_state["emitted"]] < j_hi:
                    emit_xT_group(xT_state["emitted"])
                    xT_state["emitted"] += 1

            w_bf = const.tile([128, 128], BF16)
            nc.vector.tensor_copy(w_bf, w_sb)
            ensure_xT(4)  # group 0: quad 0's rhs columns
            wt_ps = ps_work.tile([128, 512], BF16, name="t_ps", tag="pswork")
            nc.tensor.transpose(wt_ps[:, 0:128], w_bf, ident)
            wT = const.tile([128, 128], BF16)             # wT[c, o] = W[o, c]
            nc.vector.tensor_copy(wT, wt_ps[:, 0:128])

            # z~ = [x @ W.T | 1]  (bf16), tiled [j within tile, 129]
            zt = const.tile([128, NT, 129], ctx_dtype)
            nc.vector.memset(zt[:, :, 128], 1.0)

            def emit_z_group(g):
                # 8 j-tiles per group: halves the PSUM pool insertions and
                # the copy count (pool rotation during quad 0 also paces the
                # xT transposes, so fewer insertions help both)
                z_ps = ps_work.tile([128, 1024], F32, name="z_ps", tag="pswork")
                for u in range(8):
                    j = g * 8 + u
                    nc.tensor.matmul(z_ps[:, u * 128:(u + 1) * 128],
                                     xT[:, j * 128:(j + 1) * 128], wT,
                                     start=True, stop=True)
                # fold the bias into z:  sum_j E_ij (z_jo + b_o) =
                # num_io + r_i*b_o, so y = num'/r needs no bias add later
                bias_row = bass.AP(tensor=bias_bc.tensor, offset=bias_bc.offset,
                                   ap=[bias_bc.ap[0], [0, 8], bias_bc.ap[-1]])
                nc.vector.tensor_tensor(
                    zt[:, g * 8:(g + 1) * 8, 0:128],
                    z_ps.rearrange("p (j c) -> p j c", c=128),
                    bias_row, op=mybir.AluOpType.add)
                bal["d"] += (120 + 1024) / 0.96

            z_state = {"emitted": 0}

            def ensure_z(j_hi):
                need = min(NT // 8, (j_hi + 7) // 8)
                while z_state["emitted"] < need:
                    emit_z_group(z_state["emitted"])
                    z_state["emitted"] += 1

            zeros128 = const.tile([128, 128], ctx_dtype)
            nc.vector.memset(zeros128, 0.0)
            dummy258 = const.tile([128, 258], ctx_dtype)
            nc.vector.memset(dummy258, 0.0)


            # prefetch a couple of xT/z groups so quad 0's pipeline starts deep
            ensure_xT(8)
            ensure_z(4)

            # ---- main loop ----
            # Non-diagonal S slabs are written by the PE directly as BF16
            # into single-bank PSUM tiles: halves PSUM footprint (4 slabs in
            # flight) and doubles the DVE bit-trick rate (2x_1P on 16-bit
            # PSUM reads).  Diagonal j-tiles keep fp32 slabs and exact
            # ScalarE exp (~92% of softmax mass is on the diagonal).
            # The four acc accumulators pack two-per-bank: a zero matmul
            # opens the bank's accumulation group (start=True clears
            # has_written bank-wide), then every AV matmul accumulates with
            # start=False.  S-matmuls are emitted three slabs AHEAD so they
            # sit in front of in-flight AVs in the PE FIFO.
            JG = [2] * 16             # j-block group sizes per quad (sum=32);
                                      # 2-bank slabs, 3 in flight (bufs=3),
                                      # so exp latency jitter never stalls PE
            NB = QW // 128            # i-blocks per quad (4)
            groups = []
            for q in range(NQ):
                jb = 0
                for hi, gsz in enumerate(JG):
                    groups.append((q, jb, gsz, hi))
                    jb += gsz

            s_tiles = {}

            def emit_S(idx):
                q, jb, gsz, hi = groups[idx]
                ensure_xT(jb + gsz)
                s_ps = ps_work.tile([128, QW * gsz], F32, name="s_ps",
                                    tag="pswork")
                for u in range(gsz):
                    j = jb + u
                    nc.tensor.matmul(s_ps[:, u * QW:(u + 1) * QW],
                                     xT[:, j * 128:(j + 1) * 128],
                                     xT[:, q * QW:(q + 1) * QW],
                                     start=True, stop=True)
                s_tiles[idx] = s_ps

            def diag_span(q, jb, gsz):
                lo = hi = None
                for u in range(gsz):
                    j = jb + u
                    if 4 * q <= j < 4 * q + 4:
                        off = u * QW + (j - 4 * q) * 128
                        lo = off if lo is None else lo
                        hi = off + 128
                return lo, hi

            def emit_exp(s_ps, b_sb, q, jb, gsz):
                """Assign each slab's exp wholly to ScalarE (native exp) or
                wholly to the VectorE bit-trick, alternating via the running
                busy estimate (whole slabs halve per-instruction overhead vs
                always splitting); slabs containing diagonal blocks keep the
                in-slab split with ScalarE forced over the diagonal span."""
                FD = gsz * QW
                dlo, dhi = diag_span(q, jb, gsz)
                b_i16 = b_sb.bitcast(I16)
                # cap the scalar segment so its latency (~(350+FD)/1.2 ns)
                # stays inside the 882ns/slab PE period; diagonal spans may
                # exceed the cap when forced
                cap = 640 if dlo is None else max(640, dhi - dlo)
                cands = [(a, b2) for a in range(0, FD + 128, 128)
                         for b2 in range(a, FD + 128, 128)
                         if (b2 - a) <= cap
                         and (dlo is None or (a <= dlo and b2 >= dhi))]
                best = None
                for a, b2 in cands:
                    ts = bal["s"] + ((350 + (b2 - a)) / 1.2 if b2 > a else 0.0)
                    td = bal["d"] + sum((120 + (h - l)) / 0.96
                                        for l, h in ((0, a), (b2, FD)) if h > l)
                    m = max(ts, td)
                    if best is None or m < best[0]:
                        best = (m, a, b2)
                _, a, b2 = best
                if b2 > a:
                    nc.scalar.activation(b_sb[:, a:b2], s_ps[:, a:b2],
                                         mybir.ActivationFunctionType.Exp,
                                         scale=SCALE_ADJ)
                    bal["s"] += (350 + (b2 - a)) / 1.2
                for l, h in ((0, a), (b2, FD)):
                    if h > l:
                        nc.vector.tensor_scalar(b_i16[:, l:h], s_ps[:, l:h],
                                                EXP_K1, EXP_K2,
                                                op0=mybir.AluOpType.mult,
                                                op1=mybir.AluOpType.add)
                        bal["d"] += (120 + (h - l)) / 0.96

            emit_S(0)
            emit_S(1)
            acc = None
            acc_slice = None
            for idx, (q, jb, gsz, hi) in enumerate(groups):
                if hi == 0:
                    acc = [ps_acc.tile([128, 258], F32, name=f"acc{p}",
                                       tag="acc")
                           for p in range(NB // 2)]

                    def acc_slice(k, w=129, _acc=acc):
                        return _acc[k // 2][:, (k % 2) * 129:(k % 2) * 129 + w]

                if idx + 2 < len(groups):
                    emit_S(idx + 2)
                s_ps = s_tiles.pop(idx)
                b_sb = bwork.tile([128, QW * gsz], ctx_dtype, name="b_sb",
                                  tag="b_sb")
                emit_exp(s_ps, b_sb, q, jb, gsz)
                ensure_z(jb + gsz)
                for u in range(gsz):
                    j = jb + u
                    for k in range(NB):
                        # first MM into each acc bank per quad opens the
                        # accumulation group: start=True clears has_written
                        # bank-wide, the paired k (odd) then overwrites its
                        # untouched columns, and all later j accumulate.
                        # MMs complete in FIFO order, so this is safe.
                        nc.tensor.matmul(
                            acc_slice(k),
                            b_sb[:, u * QW + k * 128:u * QW + (k + 1) * 128],
                            zt[:, j, :],
                            start=(hi == 0 and u == 0 and k % 2 == 0),
                            stop=(j == NT - 1),
                            skip_group_check=True)
                if hi != len(JG) - 1:
                    continue
                # epilogue: y = acc[:, :128] / acc[:, 128]  (bias rides in z).
                # The two accs packed per bank sit at stride 129, so one
                # strided reciprocal + one tensor_tensor multiply handles a
                # pair; on the final quad half the scales run on ScalarE so
                # the drain tail isn't serialized through the DVE queue.
                y4 = ywork.tile([128, NB, 128], F32, name="y4", tag="y4")
                for p in range(NB // 2):
                    base = acc[p]
                    rinv2 = ywork.tile([128, 2], F32, name="rinv", tag="rinv")
                    r_pair = bass.AP(tensor=base.tensor, offset=base.offset + 128,
                                     ap=[base.ap[0], [129, 2]])
                    nc.vector.reciprocal(rinv2, r_pair)
                    if q == NQ - 1 and p == 1:
                        for kk in range(2):
                            nc.scalar.activation(
                                y4[:, 2 * p + kk, :], acc_slice(2 * p + kk, 128),
                                mybir.ActivationFunctionType.Copy,
                                scale=rinv2[:, kk:kk + 1])
                    else:
                        num_pair = bass.AP(tensor=base.tensor, offset=base.offset,
                                           ap=[base.ap[0], [129, 2], [1, 128]])
                        rinv_bc = bass.AP(tensor=rinv2.tensor, offset=rinv2.offset,
                                          ap=[rinv2.ap[0], [1, 2], [0, 128]])
                        nc.vector.tensor_tensor(y4[:, 2 * p:2 * p + 2, :],
                                                num_pair, rinv_bc,
                                                op=mybir.AluOpType.mult)
                bal["d"] += 1100.0
                o_view = o_d.rearrange("(p m) c -> p m c", m=NT)
                if q == NQ - 1:
                    # final quad is on the drain critical path: one DMA per
                    # i-block, striped over both HWDGE rings, so the first
                    # writes start while later epilogue ops still run
                    for k in range(NB):
                        eng = nc.sync if k % 2 == 0 else nc.scalar
                        eng.dma_start(o_view[:, q * NB + k, :], y4[:, k, :])
                else:
                    nc.sync.dma_start(o_view[:, q * NB:(q + 1) * NB, :], y4)

    nc.compile()
    return nc


_NC_CACHE = {}


def _get_nc():
    if "nc" not in _NC_CACHE:
        _NC_CACHE["nc"] = _build()
    return _NC_CACHE["nc"]


def kernel(x, W, b, _trace=False):
    """x: [8, 4096, 128] f32, W: [128, 128] f32, b: [128] f32 -> [8, 4096, 128] f32."""
    nc = _get_nc()
    x = np.ascontiguousarray(np.asarray(x, dtype=np.float32))
    W = np.ascontiguousarray(np.asarray(W, dtype=np.float32))
    b = np.ascontiguousarray(np.asarray(b, dtype=np.float32))
    in_maps = [{"x": x[i], "W": W, "b": b} for i in range(B)]
    res = bass_utils.run_bass_kernel_spmd(nc, in_maps, core_ids=list(range(B)),
                                          trace=_trace)
    out = np.stack([r["out"] for r in res.results]).astype(np.float32)
    if _trace:
        return out, res
    return out
